# revision 3
# baseline (speedup 1.0000x reference)
"""CrossAttn block kernel for 8 Trainium2 NeuronCores.

Sharding: 8 shards = batch(4) x query-half(2). Each core recomputes K/V for
its batch (10% duplicate FLOPs, zero collectives).

Device layout is fully transposed: every activation lives as [feature, seq]
in SBUF, so every matmul uses a natural-layout weight as the stationary
operand and the activation as the moving operand. Softmax runs on S^T with
partition-axis sums done by ones-matmuls; LayerNorm affine (g, b) and the
attention scale are folded into the weights on the host.

SBUF is managed as one arena pool whose tagged slots are reused across
phases (x -> W1, K -> gelu output, yln -> W2, Q -> P, V -> h, ...).

b=4, n=2048, ch=512, heads=8, dim_head=64, inner=512, mlp hidden=2048.
"""

import numpy as np
import ml_dtypes

import concourse.bacc as bacc
import concourse.tile as tile
from concourse import mybir
from concourse.bass_utils import run_bass_kernel_spmd

F32 = mybir.dt.float32
BF16 = mybir.dt.bfloat16
AF = mybir.ActivationFunctionType

B, N, CH = 4, 2048, 512
HEADS, DH = 8, 64
INNER = HEADS * DH          # 512
MLP = 4 * CH                # 2048
NQ = N // 2                 # 1024 queries per core
NCORES = 8
EPS = 1e-5
SCALE = DH ** -0.5
KT = 4                      # 512 channels = 4 k-tiles of 128
NKT = N // 128              # 16 nk tiles
NJ = MLP // 128             # 16 mlp-hidden tiles
C5 = 512

_cache = {}


def _build():
    nc = bacc.Bacc("TRN2", target_bir_lowering=False)

    xT = nc.dram_tensor("xT", [CH, N], F32, kind="ExternalInput")
    yT = nc.dram_tensor("yT", [CH, NQ], F32, kind="ExternalInput")
    wq = nc.dram_tensor("wq", [CH, INNER], F32, kind="ExternalInput")
    wk = nc.dram_tensor("wk", [CH, INNER], F32, kind="ExternalInput")
    wv = nc.dram_tensor("wv", [CH, INNER], F32, kind="ExternalInput")
    wp = nc.dram_tensor("wp", [INNER, CH], F32, kind="ExternalInput")
    w1 = nc.dram_tensor("w1", [CH, MLP], F32, kind="ExternalInput")
    w2 = nc.dram_tensor("w2", [MLP, CH], BF16, kind="ExternalInput")
    bq = nc.dram_tensor("bq", [128, 4], F32, kind="ExternalInput")
    bk = nc.dram_tensor("bk", [128, 4], F32, kind="ExternalInput")
    bv = nc.dram_tensor("bv", [1, INNER], F32, kind="ExternalInput")
    bp = nc.dram_tensor("bp", [128, 4], F32, kind="ExternalInput")
    b1 = nc.dram_tensor("b1", [128, 16], F32, kind="ExternalInput")
    b2 = nc.dram_tensor("b2", [128, 4], F32, kind="ExternalInput")
    out = nc.dram_tensor("out", [CH, NQ], F32, kind="ExternalOutput")

    with tile.TileContext(nc) as tc:
        ar = tc.alloc_tile_pool(name="ar", bufs=1)

        # ---- consts / biases ----
        ones_col = ar.tile([128, 1], F32, tag="c_oc", name="ones_col")
        nc.vector.memset(ones_col, 1.0)
        pos1 = ar.tile([1, 128], F32, tag="c_p1", name="pos1")
        nc.vector.memset(pos1, 1.0)
        neginv = ar.tile([1, 128], F32, tag="c_ni", name="neginv")
        nc.vector.memset(neginv, -1.0 / CH)
        eps_t = ar.tile([1, 1], F32, tag="c_ep", name="eps_t")
        nc.vector.memset(eps_t, EPS)
        ones_v = ar.tile([128, DH], BF16, tag="c_ov", name="ones_v")
        nc.vector.memset(ones_v, 1.0)
        bq_sb = ar.tile([128, 4], F32, tag="c_bq", name="bq_sb")
        nc.sync.dma_start(bq_sb[:], bq[:])
        bk_sb = ar.tile([128, 4], F32, tag="c_bk", name="bk_sb")
        nc.sync.dma_start(bk_sb[:], bk[:])
        bp_sb = ar.tile([128, 4], F32, tag="c_bp", name="bp_sb")
        nc.sync.dma_start(bp_sb[:], bp[:])
        b1_sb = ar.tile([128, 16], F32, tag="c_b1", name="b1_sb")
        nc.sync.dma_start(b1_sb[:], b1[:])
        b2_sb = ar.tile([128, 4], F32, tag="c_b2", name="b2_sb")
        nc.sync.dma_start(b2_sb[:], b2[:])
        bv_row = ar.tile([1, INNER], F32, tag="c_bv", name="bv_row")
        nc.sync.dma_start(bv_row[:], bv[:])
        bv_bc = ar.tile([128, INNER], F32, tag="c_bvb", name="bv_bc")

        # ---- attention weights (packed wq|wk|wv per k-tile) + x, y ----
        wa = [ar.tile([128, 3 * INNER], F32, tag=f"wa{k}", name=f"wa{k}")
              for k in range(KT)]
        wp2 = [ar.tile([128, 2 * CH], F32, tag=f"wp{i}", name=f"wp{i}")
               for i in range(2)]
        for k in range(KT):
            sl = slice(k * 128, (k + 1) * 128)
            nc.sync.dma_start(wa[k][:, 0:INNER], wq[sl, :])
            nc.sync.dma_start(wa[k][:, INNER:2 * INNER], wk[sl, :])
            nc.sync.dma_start(wa[k][:, 2 * INNER:3 * INNER], wv[sl, :])
            nc.sync.dma_start(wp2[k // 2][:, (k % 2) * CH:(k % 2 + 1) * CH],
                              wp[sl, :])
        wq_s = lambda k, msl: wa[k][:, msl]
        wk_s = lambda k, msl: wa[k][:, INNER + msl.start:INNER + msl.stop]
        wv_s = lambda k: wa[k][:, 2 * INNER:3 * INNER]
        wp_s = lambda k, msl: wp2[k // 2][:, (k % 2) * CH + msl.start:
                                          (k % 2) * CH + msl.stop]

        x_sb = [ar.tile([128, N], F32, tag=f"xa{k}", name=f"x{k}")
                for k in range(KT)]
        y_sb = [ar.tile([128, NQ], F32, tag=f"ya{k}", name=f"y{k}")
                for k in range(KT)]
        yln_sb = [ar.tile([128, NQ], F32, tag=f"qa{k}", name=f"yl{k}")
                  for k in range(KT)]
        for k in range(KT):
            sl = slice(k * 128, (k + 1) * 128)
            nc.sync.dma_start(x_sb[k][:], xT[sl, :])
            nc.sync.dma_start(y_sb[k][:], yT[sl, :])

        # ================= Phase A: LayerNorm 1 ==================
        # stats slots: x-chain in ka0..2 ([1,N]), y-chain in qb0..2 ([1,NQ])
        sx = ar.tile([1, N], F32, tag="ka0", name="sx")
        qx = ar.tile([1, N], F32, tag="ka1", name="qx")
        rx = ar.tile([1, N], F32, tag="ka2", name="rx")
        sy = ar.tile([1, NQ], F32, tag="qb0", name="sy")
        qy = ar.tile([1, NQ], F32, tag="qb1", name="qy")
        ry = ar.tile([1, NQ], F32, tag="qb2", name="ry")
        with tc.tile_pool(name="pa_ps", bufs=1, space="PSUM") as pa_ps:
            st_ps = pa_ps.tile([1, N + NQ], F32, tag="st")
            for c in range(N // C5):
                sl = slice(c * C5, (c + 1) * C5)
                for k in range(KT):
                    nc.tensor.matmul(st_ps[:, sl], ones_col[:], x_sb[k][:, sl],
                                     start=(k == 0), stop=(k == KT - 1))
            for c in range(NQ // C5):
                sl = slice(c * C5, (c + 1) * C5)
                psl = slice(N + c * C5, N + (c + 1) * C5)
                for k in range(KT):
                    nc.tensor.matmul(st_ps[:, psl], ones_col[:], y_sb[k][:, sl],
                                     start=(k == 0), stop=(k == KT - 1))
            nc.scalar.copy(sx[:], st_ps[:, 0:N])
            nc.scalar.copy(sy[:], st_ps[:, N:N + NQ])

            sq_ps = pa_ps.tile([1, N + NQ], F32, tag="st")
            for k in range(KT):
                xsq = ar.tile([128, N], F32, tag="ka3", name=f"xsq{k}")
                nc.scalar.square(xsq[:], x_sb[k][:])
                for c in range(N // C5):
                    sl = slice(c * C5, (c + 1) * C5)
                    nc.tensor.matmul(sq_ps[:, sl], ones_col[:], xsq[:, sl],
                                     start=(k == 0), stop=(k == KT - 1))
            for k in range(KT):
                ysq = ar.tile([128, NQ], F32, tag="qb3", name=f"ysq{k}")
                nc.vector.tensor_mul(ysq[:], y_sb[k][:], y_sb[k][:])
                for c in range(NQ // C5):
                    sl = slice(c * C5, (c + 1) * C5)
                    psl = slice(N + c * C5, N + (c + 1) * C5)
                    nc.tensor.matmul(sq_ps[:, psl], ones_col[:], ysq[:, sl],
                                     start=(k == 0), stop=(k == KT - 1))
            nc.scalar.copy(qx[:], sq_ps[:, 0:N])
            nc.scalar.copy(qy[:], sq_ps[:, N:N + NQ])

        # rstd = exp(-0.5*ln(E[v^2]-E[v]^2+eps)); sums <- sums*rstd
        for s_, q_, r_ in ((sx, qx, rx), (sy, qy, ry)):
            nc.scalar.activation(r_[:], s_[:], AF.Square, bias=0.0,
                                 scale=1.0 / CH)
            nc.scalar.activation(q_[:], q_[:], AF.Identity,
                                 bias=eps_t[:, 0:1], scale=1.0 / CH)
            nc.vector.tensor_sub(q_[:], q_[:], r_[:])
            nc.scalar.activation(q_[:], q_[:], AF.Ln)
            nc.scalar.activation(r_[:], q_[:], AF.Exp, bias=0.0, scale=-0.5)
            nc.vector.tensor_mul(s_[:], r_[:], s_[:])

        with tc.tile_pool(name="pbc", bufs=1, space="PSUM") as pbc:
            rs_bc = pbc.tile([128, N], F32, tag="rs")
            nm_bc = pbc.tile([128, N], F32, tag="nm")
            for c in range(N // C5):
                sl = slice(c * C5, (c + 1) * C5)
                nc.tensor.matmul(rs_bc[:, sl], pos1[:], rx[:, sl],
                                 start=True, stop=True)
                nc.tensor.matmul(nm_bc[:, sl], neginv[:], sx[:, sl],
                                 start=True, stop=True)
            for k in range(KT):
                nc.vector.tensor_mul(x_sb[k][:], x_sb[k][:], rs_bc[:])
                nc.vector.tensor_add(x_sb[k][:], x_sb[k][:], nm_bc[:])
            rs_by = pbc.tile([128, NQ], F32, tag="rs")
            nm_by = pbc.tile([128, NQ], F32, tag="nm")
            for c in range(NQ // C5):
                sl = slice(c * C5, (c + 1) * C5)
                nc.tensor.matmul(rs_by[:, sl], pos1[:], ry[:, sl],
                                 start=True, stop=True)
                nc.tensor.matmul(nm_by[:, sl], neginv[:], sy[:, sl],
                                 start=True, stop=True)
            for k in range(KT):
                nc.vector.tensor_mul(yln_sb[k][:], y_sb[k][:], rs_by[:])
                nc.vector.tensor_add(yln_sb[k][:], yln_sb[k][:], nm_by[:])
            bv_ps = pbc.tile([128, INNER], F32, tag="rs")
            nc.tensor.matmul(bv_ps[:], pos1[:], bv_row[:],
                             start=True, stop=True)
            nc.scalar.copy(bv_bc[:], bv_ps[:])

        # ================= Phase B: K^T, V, Q^T ======================
        k_sb = [ar.tile([128, N], F32, tag=f"ka{m}", name=f"k{m}")
                for m in range(KT)]
        v_sb = [ar.tile([128, 4 * INNER], BF16, tag=f"va{i}", name=f"v{i}")
                for i in range(4)]
        q_sb = [ar.tile([128, NQ], F32, tag=f"qb{m}", name=f"q{m}")
                for m in range(KT)]
        v_s = lambda s, lo, hi: v_sb[s // 4][:, (s % 4) * INNER + lo:
                                             (s % 4) * INNER + hi]
        with tc.tile_pool(name="pb_ps", bufs=1, space="PSUM") as pb_ps:
            for m in range(KT):
                msl = slice(m * 128, (m + 1) * 128)
                kps = pb_ps.tile([128, N], F32, tag="kps")
                for k in range(KT):
                    for c in range(N // C5):
                        sl = slice(c * C5, (c + 1) * C5)
                        nc.tensor.matmul(kps[:, sl], wk_s(k, msl),
                                         x_sb[k][:, sl],
                                         start=(k == 0), stop=(k == KT - 1))
                nc.vector.tensor_scalar_add(k_sb[m][:], kps[:],
                                            bk_sb[:, m:m + 1])
                qps = pb_ps.tile([128, NQ], F32, tag="qps")
                for k in range(KT):
                    for c in range(NQ // C5):
                        sl = slice(c * C5, (c + 1) * C5)
                        nc.tensor.matmul(qps[:, sl], wq_s(k, msl),
                                         yln_sb[k][:, sl],
                                         start=(k == 0), stop=(k == KT - 1))
                nc.vector.tensor_scalar_add(q_sb[m][:], qps[:],
                                            bq_sb[:, m:m + 1])
            for s in range(NKT):
                ssl = slice(s * 128, (s + 1) * 128)
                vps = pb_ps.tile([128, INNER], F32, tag="vps")
                for k in range(KT):
                    nc.tensor.matmul(vps[:], x_sb[k][:, ssl], wv_s(k),
                                     start=(k == 0), stop=(k == KT - 1))
                nc.vector.tensor_add(v_s(s, 0, INNER), vps[:], bv_bc[:])

        # ================= Phase C: attention ========================
        # W1/W2 arrive during attention, into the x / yln slots.
        w1_sb = [ar.tile([128, MLP], F32, tag=f"xa{k}", name=f"w1_{k}")
                 for k in range(KT)]
        w2_sb = [ar.tile([128, 4 * CH], BF16, tag=f"qa{i}", name=f"w2_{i}")
                 for i in range(4)]
        for k in range(KT):
            nc.sync.dma_start(w1_sb[k][:], w1[k * 128:(k + 1) * 128, :])
        for i in range(4):
            for j4 in range(4):
                j = i * 4 + j4
                nc.sync.dma_start(w2_sb[i][:, j4 * CH:(j4 + 1) * CH],
                                  w2[j * 128:(j + 1) * 128, :])
        w2_s = lambda j, msl: w2_sb[j // 4][:, (j % 4) * CH + msl.start:
                                            (j % 4) * CH + msl.stop]

        o_sb = [ar.tile([128, NQ], F32, tag=f"oa{p}", name=f"o{p}")
                for p in range(4)]
        with tc.tile_pool(name="pc_ps", bufs=1, space="PSUM") as pc_ps:
            for p in range(4):  # head pairs (2p, 2p+1)
                ha, hb = 2 * p, 2 * p + 1
                o_ps = pc_ps.tile([128, NQ], F32, tag="o_ps")
                r_ps = pc_ps.tile([128, NQ], F32, tag="r_ps")
                for s in range(NKT):
                    ssl = slice(s * 128, (s + 1) * 128)
                    s_ps = pc_ps.tile([128, 2 * NQ], F32, tag="s_ps")
                    for c in range(NQ // C5):
                        sl = slice(c * C5, (c + 1) * C5)
                        sl_b = slice(NQ + c * C5, NQ + (c + 1) * C5)
                        nc.tensor.matmul(s_ps[:, sl],
                                         k_sb[p][0:64, ssl],
                                         q_sb[p][0:64, sl],
                                         start=True, stop=True)
                        nc.tensor.matmul(s_ps[:, sl_b],
                                         k_sb[p][64:128, ssl],
                                         q_sb[p][64:128, sl],
                                         start=True, stop=True)
                    es = ar.tile([128, 2 * NQ], BF16, tag=f"wa{s % 2}",
                                 name=f"es{p}_{s}")
                    nc.scalar.activation(es[:], s_ps[:], AF.Exp)
                    for c in range(NQ // C5):
                        sl = slice(c * C5, (c + 1) * C5)
                        sl_b = slice(NQ + c * C5, NQ + (c + 1) * C5)
                        va = v_s(s, ha * DH, (ha + 1) * DH)
                        vb = v_s(s, hb * DH, (hb + 1) * DH)
                        st, sp = (s == 0), (s == NKT - 1)
                        nc.tensor.matmul(o_ps[0:64, sl], va, es[:, sl],
                                         start=st, stop=sp)
                        nc.tensor.matmul(o_ps[64:128, sl], vb, es[:, sl_b],
                                         start=st, stop=sp,
                                         tile_position=(0, 64))
                        nc.tensor.matmul(r_ps[0:64, sl], ones_v[:],
                                         es[:, sl], start=st, stop=sp)
                        nc.tensor.matmul(r_ps[64:128, sl], ones_v[:],
                                         es[:, sl_b], start=st, stop=sp,
                                         tile_position=(0, 64))
                # normalize: o = o_ps * exp(-ln(r))
                rt = ar.tile([128, NQ], F32, tag="wa3", name=f"rt{p}")
                nc.scalar.activation(rt[:], r_ps[:], AF.Ln)
                nc.scalar.activation(rt[:], rt[:], AF.Exp, bias=0.0,
                                     scale=-1.0)
                nc.vector.tensor_mul(o_sb[p][:], o_ps[:], rt[:])

        # ============ Phase D: proj + residual + LN2 =================
        p_sb = [ar.tile([128, NQ], F32, tag=f"qb{m}", name=f"p{m}")
                for m in range(KT)]
        h_sb = [ar.tile([128, NQ], F32, tag=f"va{m}", name=f"h{m}")
                for m in range(KT)]
        s2 = ar.tile([1, NQ], F32, tag="wa0", name="s2")
        q2 = ar.tile([1, NQ], F32, tag="wa1", name="q2")
        r2 = ar.tile([1, NQ], F32, tag="wa2", name="r2")
        with tc.tile_pool(name="pd_ps", bufs=1, space="PSUM") as pd_ps:
            for m in range(KT):
                msl = slice(m * 128, (m + 1) * 128)
                pps = pd_ps.tile([128, NQ], F32, tag="pps")
                for k in range(KT):
                    for c in range(NQ // C5):
                        sl = slice(c * C5, (c + 1) * C5)
                        nc.tensor.matmul(pps[:, sl], wp_s(k, msl),
                                         o_sb[k][:, sl],
                                         start=(k == 0), stop=(k == KT - 1))
                nc.scalar.activation(p_sb[m][:], pps[:], AF.Identity,
                                     bias=bp_sb[:, m:m + 1])
                nc.vector.tensor_add(p_sb[m][:], p_sb[m][:], y_sb[m][:])

            s2_ps = pd_ps.tile([1, NQ], F32, tag="s2ps")
            for c in range(NQ // C5):
                sl = slice(c * C5, (c + 1) * C5)
                for m in range(KT):
                    nc.tensor.matmul(s2_ps[:, sl], ones_col[:],
                                     p_sb[m][:, sl],
                                     start=(m == 0), stop=(m == KT - 1))
            nc.scalar.copy(s2[:], s2_ps[:])
            q2_ps = pd_ps.tile([1, NQ], F32, tag="s2ps")
            for m in range(KT):
                psq = ar.tile([128, NQ], F32, tag=f"wp{m % 2}",
                              name=f"psq{m}")
                nc.scalar.square(psq[:], p_sb[m][:])
                for c in range(NQ // C5):
                    sl = slice(c * C5, (c + 1) * C5)
                    nc.tensor.matmul(q2_ps[:, sl], ones_col[:], psq[:, sl],
                                     start=(m == 0), stop=(m == KT - 1))
            nc.scalar.copy(q2[:], q2_ps[:])

            nc.scalar.activation(r2[:], s2[:], AF.Square, bias=0.0,
                                 scale=1.0 / CH)
            nc.scalar.activation(q2[:], q2[:], AF.Identity,
                                 bias=eps_t[:, 0:1], scale=1.0 / CH)
            nc.vector.tensor_sub(q2[:], q2[:], r2[:])
            nc.scalar.activation(q2[:], q2[:], AF.Ln)
            nc.scalar.activation(r2[:], q2[:], AF.Exp, bias=0.0, scale=-0.5)
            nc.vector.tensor_mul(s2[:], r2[:], s2[:])

            rs2_bc = pd_ps.tile([128, NQ], F32, tag="pps")
            nm2_bc = pd_ps.tile([128, NQ], F32, tag="nm2")
            for c in range(NQ // C5):
                sl = slice(c * C5, (c + 1) * C5)
                nc.tensor.matmul(rs2_bc[:, sl], pos1[:], r2[:, sl],
                                 start=True, stop=True)
                nc.tensor.matmul(nm2_bc[:, sl], neginv[:], s2[:, sl],
                                 start=True, stop=True)
            for m in range(KT):
                nc.vector.tensor_mul(h_sb[m][:], p_sb[m][:], rs2_bc[:])
                nc.vector.tensor_add(h_sb[m][:], h_sb[m][:], nm2_bc[:])

        # ================= Phase E: MLP ==============================
        g_sb = [ar.tile([128, 4 * NQ], BF16, tag=f"ka{i}", name=f"g{i}")
                for i in range(4)]
        g_s = lambda j, lo, hi: g_sb[j // 4][:, (j % 4) * NQ + lo:
                                             (j % 4) * NQ + hi]
        with tc.tile_pool(name="pe_ps1", bufs=2, space="PSUM") as pe_ps1:
            for j in range(NJ):
                jsl = slice(j * 128, (j + 1) * 128)
                m1 = pe_ps1.tile([128, NQ], F32, tag="m1")
                for k in range(KT):
                    for c in range(NQ // C5):
                        sl = slice(c * C5, (c + 1) * C5)
                        nc.tensor.matmul(m1[:, sl], w1_sb[k][:, jsl],
                                         h_sb[k][:, sl],
                                         start=(k == 0), stop=(k == KT - 1))
                nc.scalar.activation(g_s(j, 0, NQ), m1[:], AF.Gelu_apprx_tanh,
                                     bias=b1_sb[:, j:j + 1])

        with tc.tile_pool(name="pe_ps2", bufs=2, space="PSUM") as pe_ps2:
            for m in range(KT):
                msl = slice(m * 128, (m + 1) * 128)
                ops = pe_ps2.tile([128, NQ], F32, tag="ops")
                for j in range(NJ):
                    for c in range(NQ // C5):
                        sl = slice(c * C5, (c + 1) * C5)
                        nc.tensor.matmul(ops[:, sl], w2_s(j, msl),
                                         g_s(j, c * C5, (c + 1) * C5),
                                         start=(j == 0), stop=(j == NJ - 1))
                ot = ar.tile([128, NQ], F32, tag=f"oa{m}", name=f"ot{m}")
                nc.scalar.activation(ot[:], ops[:], AF.Identity,
                                     bias=b2_sb[:, m:m + 1])
                nc.vector.tensor_add(ot[:], ot[:], p_sb[m][:])
                nc.sync.dma_start(out[msl, :], ot[:])

        ar.release()

    nc.finalize()
    return nc


def _prep(inputs):
    x = np.asarray(inputs["x"], np.float32)
    y = np.asarray(inputs["y"], np.float32)
    Wq = np.asarray(inputs["Wq"], np.float32)
    Wkv = np.asarray(inputs["Wkv"], np.float32)
    Wp = np.asarray(inputs["Wp"], np.float32)
    bp = np.asarray(inputs["bp"], np.float32)
    W1 = np.asarray(inputs["W1"], np.float32)
    b1 = np.asarray(inputs["b1"], np.float32)
    W2 = np.asarray(inputs["W2"], np.float32)
    b2 = np.asarray(inputs["b2"], np.float32)
    g1y = np.asarray(inputs["g_ln1y"], np.float32)
    b1y = np.asarray(inputs["b_ln1y"], np.float32)
    g1x = np.asarray(inputs["g_ln1x"], np.float32)
    b1x = np.asarray(inputs["b_ln1x"], np.float32)
    g2 = np.asarray(inputs["g_ln2"], np.float32)
    b2l = np.asarray(inputs["b_ln2"], np.float32)

    wq_f = (g1y[:, None] * Wq * SCALE).astype(np.float32)
    bq_f = (b1y @ Wq * SCALE).astype(np.float32)
    wk_f = (g1x[:, None] * Wkv[:, :INNER]).astype(np.float32)
    bk_f = (b1x @ Wkv[:, :INNER]).astype(np.float32)
    wv_f = (g1x[:, None] * Wkv[:, INNER:]).astype(np.float32)
    bv_f = (b1x @ Wkv[:, INNER:]).astype(np.float32)
    w1_f = (g2[:, None] * W1).astype(np.float32)
    b1_f = (b1 + b2l @ W1).astype(np.float32)

    shared = {
        "wq": np.ascontiguousarray(wq_f),
        "wk": np.ascontiguousarray(wk_f),
        "wv": np.ascontiguousarray(wv_f),
        "wp": np.ascontiguousarray(Wp),
        "w1": np.ascontiguousarray(w1_f),
        "w2": np.ascontiguousarray(W2.astype(ml_dtypes.bfloat16)),
        "bq": np.ascontiguousarray(bq_f.reshape(4, 128).T),
        "bk": np.ascontiguousarray(bk_f.reshape(4, 128).T),
        "bv": np.ascontiguousarray(bv_f.reshape(1, INNER)),
        "bp": np.ascontiguousarray(bp.reshape(4, 128).T),
        "b1": np.ascontiguousarray(b1_f.reshape(16, 128).T),
        "b2": np.ascontiguousarray(b2.reshape(4, 128).T),
    }
    xT = np.ascontiguousarray(x.transpose(0, 2, 1))   # [B, CH, N]
    yT = np.ascontiguousarray(y.transpose(0, 2, 1))   # [B, CH, N]
    in_maps = []
    for c in range(NCORES):
        bi, qh = c // 2, c % 2
        m = dict(shared)
        m["xT"] = xT[bi]
        m["yT"] = np.ascontiguousarray(yT[bi][:, qh * NQ:(qh + 1) * NQ])
        in_maps.append(m)
    return in_maps


def kernel(**inputs):
    if "nc" not in _cache:
        _cache["nc"] = _build()
    nc = _cache["nc"]
    in_maps = _prep(inputs)
    res = run_bass_kernel_spmd(nc, in_maps, core_ids=list(range(NCORES)))
    out = np.empty((B, N, CH), np.float32)
    for c in range(NCORES):
        bi, qh = c // 2, c % 2
        out[bi, qh * NQ:(qh + 1) * NQ, :] = res.results[c]["out"].T
    return out
